# revision 27
# baseline (speedup 1.0000x reference)
"""AugmentedMamba3 — Bass/Tile kernel for 8 Trainium2 NeuronCores.

Sharding: core i = (batch b = i//2, half hf = i%2); each core owns T=1024
tokens of one batch element.  The sequential scan is a linear recurrence in
the register/memory state, so it is computed chunk-wise (8 chunks of 128
tokens): per-chunk projections + causal 128x128 attention-style blocks plus
a tiny sequential state accumulation.

Second-half cores rebuild the incoming state from the first half using
linearity:  reg_init = (A_prev^T @ u_prev) @ W_val^T + colsum(A_prev) x b,
which needs only rank-8/16 reductions of u_prev — no big recompute and no
cross-core communication.

All GEMMs run in bf16 (fp32 PSUM accumulation); softmax/layernorm/state
masters in fp32.  Everything is hardcoded for B=4, L=2048, D=1024.
"""

import sys

sys.path.insert(0, "/opt/trn_rl_repo")

from contextlib import ExitStack

import ml_dtypes
import numpy as np

import concourse.bass as bass
import concourse.bacc as bacc
import concourse.tile as tile
from concourse import mybir
from concourse.masks import make_identity

BF = ml_dtypes.bfloat16
E4 = ml_dtypes.float8_e4m3
F32 = np.float32

B, L, D = 4, 2048, 1024
T = 1024          # tokens per core
P = 128           # chunk / partition size
NCH = T // P      # 8 token chunks
ND = D // P       # 8 feature chunks
NREG, NMEM = 8, 16
DECAY = 0.995
SHARP = 5.0
SCALE = float(D) ** -0.5
D128 = float(DECAY ** P)

f32 = mybir.dt.float32
bf16 = mybir.dt.bfloat16


def _dt(np_dtype):
    if np_dtype == BF:
        return bf16
    if np_dtype == E4:
        return mybir.dt.float8e4
    return f32


# ---------------------------------------------------------------- input specs
IN_SPECS = [
    # per-core activations
    ("uT", (D, T), BF),        # own u, feature-major (u.T)
    ("uprevT", (D, T), BF),    # prev-half u, feature-major (zeros on hf=0)
    ("uprev", (T, D), BF),     # prev-half u, token-major
    # weights (host pre-transposed; *q* scaled by SCALE)
    ("wrvT", (D, D), BF),
    ("wrqT", (D, D), BF),
    ("wmvT", (D, D), BF),
    ("wmqT", (D, D), BF),
    ("whIT", (D, D), BF),      # (W_h + I).T  — residual folded in
    ("wr8", (D, D), E4),   # (W_r.T * 16) in fp8e4
    ("wm8", (D, D), E4),   # (W_m.T * 16) in fp8e4
    ("wcatT", (D, 26), BF),    # [reg_gate; reg_addr(8); mem_gate; mem_addr(16)].T
    ("bcat", (1, 26), BF),
    ("bvq", (1, 4 * D), BF),   # [b_rv, b_rq*SCALE, b_mv, b_mq*SCALE]
    ("combb", (1, D), BF),
    ("lng", (1, D), BF),
    ("lnb", (1, D), BF),
    # constants
    ("maskUT", (P, P), F32),   # 1 if t' <= t
    ("mdec", (P, P), F32),     # maskUT * DECAY^(t-t')
    ("mdec2", (P, P), F32),    # maskUT * DECAY^(-t'-1)
    ("dpow", (P, 1), F32),     # DECAY^(t+1)
    ("decvec", (P, 1), F32),   # DECAY^(127-t)
    ("wdecprev", (P, NCH), F32),  # is2 * DECAY^(1023-(c*128+t))
    ("prevmask", (P, 1), F32),    # is2
]

AF = mybir.ActivationFunctionType
OP = mybir.AluOpType
AX = mybir.AxisListType


def _bcast(ap, p=P):
    """(1, N) AP -> (p, N) AP with zero partition stride (DMA broadcast)."""
    return bass.AP(tensor=ap.tensor, offset=ap.offset,
                   ap=[[0, p]] + [list(x) for x in ap.ap[1:]])


def build_tile_kernel(ctx: ExitStack, tc: tile.TileContext, outs, ins,
                      zbias=False, zcombb=False, zln=False):
    nc = tc.nc
    out_r3 = outs["out"].rearrange("(n p) d -> p n d", p=P)

    def r3(name):
        return ins[name].rearrange("(n p) d -> p n d", p=P)

    # ------------------------------------------------------------- pools
    wgt = ctx.enter_context(tc.tile_pool(name="wgt", bufs=1))
    pers = ctx.enter_context(tc.tile_pool(name="pers", bufs=1))
    act = ctx.enter_context(tc.tile_pool(name="act", bufs=2))
    sb = ctx.enter_context(tc.tile_pool(name="sb", bufs=2))
    rd = ctx.enter_context(tc.tile_pool(name="rd", bufs=3))
    pg = ctx.enter_context(tc.tile_pool(name="pg", bufs=2, space="PSUM"))
    po = ctx.enter_context(tc.tile_pool(name="po", bufs=3, space="PSUM"))
    ps = ctx.enter_context(tc.tile_pool(name="ps", bufs=3, space="PSUM"))

    def sbt(name, shape, dtype=bf16, pool=None, tag=None):
        return (pool or pers).tile(list(shape), dtype, tag=tag or name,
                                   name=name)

    def load(name, shape, dtype=bf16, pool=None, src=None):
        t = sbt(name, shape, dtype, pool=pool or wgt)
        nc.sync.dma_start(t, src if src is not None else ins[name])
        return t

    # ------------------------------------------------------------- constants
    ident = sbt("ident", (P, P), bf16, pool=wgt)
    make_identity(nc, ident)
    maskUT = load("maskUT", (P, P), f32)
    mdec = load("mdec", (P, P), f32)
    mdec2 = load("mdec2", (P, P), f32)
    dpow_d = load("dpow", (P, 1), f32)
    decvec_d = load("decvec", (P, 1), f32)
    wdecprev_d = load("wdecprev", (P, NCH), f32)
    prevmask_d = load("prevmask", (P, 1), f32)
    # DVE copies of DMA'd scalar vectors: consumers then depend on DVE
    # (same-engine, elidable) instead of a DMA queue — keeps embedded
    # sync-wait counts within the TS-struct limit.
    dpow = sbt("dpow_v", (P, 1), f32, pool=wgt)
    nc.vector.tensor_copy(dpow, dpow_d)
    decvec = sbt("decvec_v", (P, 1), f32, pool=wgt)
    nc.vector.tensor_copy(decvec, decvec_d)
    wdecprev = sbt("wdecprev_v", (P, NCH), f32, pool=wgt)
    nc.vector.tensor_copy(wdecprev, wdecprev_d)
    prevmask = sbt("prevmask_v", (P, 1), f32, pool=wgt)
    nc.vector.tensor_copy(prevmask, prevmask_d)
    bcatw = load("bcat", (1, 26), bf16)
    bvq = load("bvq", (1, 4 * D), bf16)
    combb = load("combb", (1, D), bf16)
    lng_rep = sbt("lng_rep", (P, D), bf16, pool=wgt)
    nc.sync.dma_start(lng_rep, _bcast(ins["lng"]))
    lnb_rep = sbt("lnb_rep", (P, D), bf16, pool=wgt)
    nc.sync.dma_start(lnb_rep, _bcast(ins["lnb"]))
    ones_r = sbt("ones_r", (1, 512), bf16, pool=wgt)
    nc.vector.memset(ones_r, 1.0)
    ones_c = sbt("ones_c", (P, 1), bf16, pool=wgt)
    nc.vector.memset(ones_c, 1.0)

    wcat = load("wcat", (P, ND, 26), bf16, src=r3("wcatT"))

    # ------------------------------------------------------------- helpers
    def mm(out, lhsT, rhs, start, stop, pm=None):
        nc.tensor.matmul(out, lhsT, rhs, start=start, stop=stop,
                         perf_mode=pm)

    def spike_addrs(a_ps):
        """a_ps: (P, 26) psum [gate_r, addr_r(8), gate_m, addr_m(16)].
        Returns A_r (P,8) bf16, A_m (P,16) bf16 (gate * softmax)."""
        rg = sbt("spk_rg", (P, 1), f32, pool=sb)
        nc.scalar.activation(rg, a_ps[:, 0:1], AF.Sigmoid, scale=SHARP)
        mg = sbt("spk_mg", (P, 1), f32, pool=sb)
        nc.scalar.activation(mg, a_ps[:, 9:10], AF.Sigmoid, scale=SHARP)
        res = []
        for nm, sl, gate, n in (("spk_Ar", slice(1, 9), rg, NREG),
                                ("spk_Am", slice(10, 26), mg, NMEM)):
            nmax = sbt(nm + "nx", (P, 1), f32, pool=sb)
            nc.vector.reduce_max(nmax, a_ps[:, sl], axis=AX.X, negate=True)
            ex = sbt(nm + "ex", (P, n), f32, pool=sb)
            ssum = sbt(nm + "ss", (P, 1), f32, pool=sb)
            nc.scalar.activation(ex, a_ps[:, sl], AF.Exp, bias=nmax,
                                 accum_out=ssum)
            rec = sbt(nm + "rc", (P, 1), f32, pool=sb)
            nc.vector.reciprocal(rec, ssum)
            a = sbt(nm, (P, n), bf16, pool=sb)
            nc.vector.tensor_scalar(a, ex, rec, gate, op0=OP.mult, op1=OP.mult)
            res.append(a)
        return res

    def addr_psum(xTc, apool=None, atag="ps"):
        """gate/addr logits for one token chunk of feature-major xTc
        (xTc: [P, ND, P])."""
        a_ps = (apool or ps).tile([P, 32], f32, tag=atag)
        for dc in range(ND):
            mm(a_ps[:, 0:26], xTc[:, dc, :], wcat[:, dc, :],
               start=(dc == 0), stop=False)
        mm(a_ps[:, 0:26], ones_r[0:1, 0:P], bcatw[0:1, :], start=False,
           stop=True)
        return a_ps

    # ------------------------------------------------------------- init state
    # masters (fp32) + bf16 working copies
    Cr = sbt("Cr", (NREG, D), f32)
    CrT = sbt("CrT", (P, ND, NREG), f32)
    Cm = sbt("Cm", (NMEM, D), f32)
    CmT = sbt("CmT", (P, ND, NMEM), f32)
    Cr_bf = sbt("Cr_bf", (NREG, D), bf16)
    CrT_bf = sbt("CrT_bf", (P, ND, NREG), bf16)
    Cm_bf = sbt("Cm_bf", (NMEM, D), bf16)
    CmT_bf = sbt("CmT_bf", (P, ND, NMEM), bf16)

    with tc.tile_pool(name="prev", bufs=1) as pv:
        uprevT = load("uprevT", (P, ND, T), bf16, pool=pv, src=r3("uprevT"))
        uprev = load("uprev", (P, NCH, D), bf16, pool=pv, src=r3("uprev"))
        # val weights issued after the prev-half tensors: init's addr
        # matmuls need uprevT first; wrv/wmv only at the init tail.
        wrv = load("wrv", (P, ND, D), bf16, src=r3("wrvT"))
        wmv = load("wmv", (P, ND, D), bf16, src=r3("wmvT"))

        YrT = sbt("YrT", (P, ND, NREG), f32, pool=pv)
        nc.vector.memset(YrT, 0.0)
        YmT = sbt("YmT", (P, ND, NMEM), f32, pool=pv)
        nc.vector.memset(YmT, 0.0)
        sS = sbt("sS", (1, 32), f32, pool=pv)
        nc.vector.memset(sS, 0.0)

        for c in range(NCH):
            a_ps = addr_psum(uprevT[:, :, c * P:(c + 1) * P],
                             apool=po, atag="po")
            A_rp, A_mp = spike_addrs(a_ps)
            A_rpm = sbt("A_rpm", (P, NREG), bf16, pool=sb)
            nc.vector.tensor_scalar_mul(A_rpm, A_rp, prevmask[:, 0:1])
            A_mpd = sbt("A_mpd", (P, NMEM), bf16, pool=sb)
            nc.vector.tensor_scalar_mul(A_mpd, A_mp, wdecprev[:, c:c + 1])

            y_ps = ps.tile([P, ND, NREG + NMEM], f32, tag="ps")
            for dc in range(ND):
                mm(y_ps[:, dc, 0:NREG], uprev[:, c, dc * P:(dc + 1) * P],
                   A_rpm, start=True, stop=True)
                mm(y_ps[:, dc, NREG:NREG + NMEM],
                   uprev[:, c, dc * P:(dc + 1) * P], A_mpd,
                   start=True, stop=True)
            nc.vector.tensor_add(YrT, YrT, y_ps[:, :, 0:NREG])
            nc.vector.tensor_add(YmT, YmT, y_ps[:, :, NREG:NREG + NMEM])
            if not zbias:
                s_ps = ps.tile([1, 32], f32, tag="ps")
                mm(s_ps[0:1, 0:NREG], ones_c, A_rpm, start=True, stop=True)
                mm(s_ps[0:1, NREG:NREG + NMEM], ones_c, A_mpd, start=True,
                   stop=True)
                nc.vector.tensor_add(sS[0:1, 0:24], sS[0:1, 0:24],
                                     s_ps[0:1, 0:24])

        YrT_bf = sbt("YrT_bf", (P, ND, NREG), bf16, pool=pv)
        nc.vector.tensor_copy(YrT_bf, YrT)
        YmT_bf = sbt("YmT_bf", (P, ND, NMEM), bf16, pool=pv)
        nc.vector.tensor_copy(YmT_bf, YmT)
        sS_bf = sbt("sS_bf", (1, 32), bf16, pool=pv)
        nc.vector.tensor_copy(sS_bf, sS)

        for (Cx, CxT, Yb, sSl, wv, brow, n) in (
                (Cr, CrT, YrT_bf, slice(0, NREG), wrv, 0, NREG),
                (Cm, CmT, YmT_bf, slice(NREG, NREG + NMEM), wmv, 2, NMEM)):
            for jc in range(2):
                jsl = slice(jc * 512, (jc + 1) * 512)
                cps = po.tile([n, 512], f32, tag="po")
                for dc in range(ND):
                    mm(cps, Yb[:, dc, :], wv[:, dc, jsl], start=(dc == 0),
                       stop=(zbias and dc == ND - 1))
                if not zbias:
                    mm(cps, sS_bf[0:1, sSl],
                       bvq[0:1, brow * D + jc * 512:brow * D + (jc + 1) * 512],
                       start=False, stop=True)
                nc.vector.tensor_copy(Cx[:, jsl], cps)
            for jd in range(ND):
                jsl = slice(jd * P, (jd + 1) * P)
                tps = ps.tile([P, n], f32, tag="ps")
                for dc in range(ND):
                    mm(tps, wv[:, dc, jsl], Yb[:, dc, :], start=(dc == 0),
                       stop=(zbias and dc == ND - 1))
                if not zbias:
                    mm(tps, bvq[0:1, brow * D + jd * P:brow * D + (jd + 1) * P],
                       sS_bf[0:1, sSl], start=False, stop=True)
                nc.vector.tensor_copy(CxT[:, jd, :], tps)

    nc.vector.tensor_copy(Cr_bf, Cr)
    nc.vector.tensor_copy(CrT_bf, CrT)
    nc.vector.tensor_copy(Cm_bf, Cm)
    nc.vector.tensor_copy(CmT_bf, CmT)

    # remaining weights (pool opened after `prev` closes so the space is free)
    wgt2 = ctx.enter_context(tc.tile_pool(name="wgt2", bufs=1))
    wrq = load("wrq", (P, ND, D), bf16, pool=wgt2, src=r3("wrqT"))
    wmq = load("wmq", (P, ND, D), bf16, pool=wgt2, src=r3("wmqT"))
    whI = load("whI", (P, ND, D), bf16, pool=wgt2, src=r3("whIT"))
    f8 = mybir.dt.float8e4
    wr_ = load("wr_", (P, ND, D), f8, pool=wgt2, src=r3("wr8"))
    wm_ = load("wm_", (P, ND, D), f8, pool=wgt2, src=r3("wm8"))

    # ------------------------------------------------------------- chunk loop
    uT_dram = r3("uT")

    def proj_tm(specs, u_c):
        """token-major projections for one chunk; specs = [(dst, wT, brow)].
        Paired so consecutive matmuls share the stationary lhsT tile."""
        for jc in range(2):
            jsl = slice(jc * 512, (jc + 1) * 512)
            gs = [po.tile([P, 512], f32, tag="po", name="g%d" % gi)
                  for gi in range(len(specs))]
            for dc in range(ND):
                for g, (dst, wT, brow) in zip(gs, specs):
                    mm(g, u_c[:, dc, :], wT[:, dc, jsl], start=(dc == 0),
                       stop=(zbias and dc == ND - 1))
            for g, (dst, wT, brow) in zip(gs, specs):
                if not zbias:
                    mm(g, ones_r[0:1, 0:P],
                       bvq[0:1, brow * D + jc * 512:brow * D + (jc + 1) * 512],
                       start=False, stop=True)
                nc.vector.tensor_copy(dst[:, jsl], g)

    def transp8(dst, src_tm):
        """dst [P, ND, P] bf16 (feature-major) = per-128-block transpose of
        src_tm [P, D] bf16 (token-major).  4 transposes share one PSUM
        bank (bf16 128x128 = 256B/partition) -> deeper PE pipelining and
        4x fewer DVE evictions."""
        for q in range(ND // 4):
            t_ps = ps.tile([P, 4, P], bf16, tag="ps", name="tp%d" % q)
            for i in range(4):
                dc = 4 * q + i
                nc.tensor.transpose(t_ps[:, i, :],
                                    src_tm[:, dc * P:(dc + 1) * P], ident)
            nc.vector.tensor_copy(dst[:, 4 * q:4 * q + 4, :], t_ps)

    for c in range(NCH):
        csl = slice(c * P, (c + 1) * P)
        u_c = act.tile([P, ND, P], bf16, tag="u_c", name="u_c%d" % c, bufs=3)
        nc.sync.dma_start(u_c, uT_dram[:, :, csl])

        # per-chunk projections (token-major) + PE transposes (feature-major)
        rv_c = act.tile([P, D], bf16, tag="rv_c", bufs=3)
        mv_c = act.tile([P, D], bf16, tag="mv_c", bufs=3)
        proj_tm([(rv_c, wrv, 0), (mv_c, wmv, 2)], u_c)
        rq_c = act.tile([P, D], bf16, tag="rq_c", bufs=3)
        mq_c = act.tile([P, D], bf16, tag="mq_c", bufs=3)
        proj_tm([(rq_c, wrq, 1), (mq_c, wmq, 3)], u_c)
        rqT_c = act.tile([P, ND, P], bf16, tag="rqT_c", bufs=3)
        transp8(rqT_c, rq_c)
        mqT_c = act.tile([P, ND, P], bf16, tag="mqT_c", bufs=3)
        transp8(mqT_c, mq_c)
        rvT_c = act.tile([P, ND, P], bf16, tag="rvT_c", bufs=3)
        transp8(rvT_c, rv_c)
        mvT_c = act.tile([P, ND, P], bf16, tag="mvT_c", bufs=3)
        transp8(mvT_c, mv_c)

        # own gate/addr
        a_ps = addr_psum(u_c)
        A_r, A_m = spike_addrs(a_ps)
        A_md = sbt("A_md", (P, NMEM), bf16, pool=sb)
        nc.vector.tensor_scalar_mul(A_md, A_m, decvec[:, 0:1])

        art_ps = ps.tile([NREG, P], bf16, tag="ps")
        nc.tensor.transpose(art_ps, A_r, ident)
        A_rT = sbt("A_rT", (NREG, P), bf16, pool=sb)
        nc.vector.tensor_copy(A_rT, art_ps)
        amt_ps = ps.tile([NMEM, P], bf16, tag="ps")
        nc.tensor.transpose(amt_ps, A_m, ident)
        A_mT = sbt("A_mT", (NMEM, P), bf16, pool=sb)
        nc.vector.tensor_copy(A_mT, amt_ps)

        # ---------------- register bank
        gt_ps = pg.tile([P, P], f32, tag="pg")
        for dc in range(ND):
            mm(gt_ps, rvT_c[:, dc, :], rqT_c[:, dc, :], start=(dc == 0),
               stop=(dc == ND - 1))
        GTm = sbt("GTm", (P, P), bf16, pool=sb)
        nc.vector.tensor_mul(GTm, gt_ps, maskUT)

        sc_ps = ps.tile([P, NREG], f32, tag="ps")
        mm(sc_ps, GTm, A_r, start=True, stop=False)
        for dc in range(ND):
            mm(sc_ps, rqT_c[:, dc, :], CrT_bf[:, dc, :], start=False,
               stop=(dc == ND - 1))
        nmax = sbt("rnx", (P, 1), f32, pool=sb)
        nc.vector.reduce_max(nmax, sc_ps, axis=AX.X, negate=True)
        ex = sbt("rex", (P, NREG), f32, pool=sb)
        ssum = sbt("rss", (P, 1), f32, pool=sb)
        nc.scalar.activation(ex, sc_ps, AF.Exp, bias=nmax, accum_out=ssum)
        rec = sbt("rrc", (P, 1), f32, pool=sb)
        nc.vector.reciprocal(rec, ssum)
        P_r = sbt("P_r", (P, NREG), bf16, pool=sb)
        nc.vector.tensor_scalar_mul(P_r, ex, rec)

        pt_ps = ps.tile([NREG, P], bf16, tag="ps")
        nc.tensor.transpose(pt_ps, P_r, ident)
        PT = sbt("PT", (NREG, P), bf16, pool=sb)
        nc.vector.tensor_copy(PT, pt_ps)

        wt_ps = pg.tile([P, P], f32, tag="pg")
        mm(wt_ps, A_rT, PT, start=True, stop=True)
        WTm = sbt("WTm", (P, P), bf16, pool=sb)
        nc.vector.tensor_mul(WTm, wt_ps, maskUT)

        RT = rd.tile([P, ND, P], mybir.dt.float8e4, tag="RT")
        for q in range(ND // 4):
            r_ps = pg.tile([P, 4, P], f32, tag="pg", name="rps%d" % q)
            for i in range(4):
                dc = 4 * q + i
                mm(r_ps[:, i, :], rv_c[:, dc * P:(dc + 1) * P], WTm,
                   start=True, stop=False)
                mm(r_ps[:, i, :], Cr_bf[0:NREG, dc * P:(dc + 1) * P], PT,
                   start=False, stop=True)
            nc.vector.tensor_scalar_mul(RT[:, 4 * q:4 * q + 4, :], r_ps,
                                        1.0 / 16.0)

        # ---------------- memory bank
        gtm_ps = pg.tile([P, P], f32, tag="pg")
        for dc in range(ND):
            mm(gtm_ps, mvT_c[:, dc, :], mqT_c[:, dc, :], start=(dc == 0),
               stop=(dc == ND - 1))
        GTmM = sbt("GTmM", (P, P), bf16, pool=sb)
        nc.vector.tensor_mul(GTmM, gtm_ps, mdec)

        scm_ps = ps.tile([P, NMEM], f32, tag="ps")
        mm(scm_ps, GTmM, A_m, start=True, stop=True)
        sci_ps = ps.tile([P, NMEM], f32, tag="ps")
        for dc in range(ND):
            mm(sci_ps, mqT_c[:, dc, :], CmT_bf[:, dc, :], start=(dc == 0),
               stop=(dc == ND - 1))
        scm_i = sbt("scm_i", (P, NMEM), f32, pool=sb)
        nc.vector.tensor_scalar_mul(scm_i, sci_ps, dpow[:, 0:1])
        scm = sbt("scm", (P, NMEM), f32, pool=sb)
        nc.vector.tensor_add(scm, scm_i, scm_ps)
        nmaxm = sbt("mnx", (P, 1), f32, pool=sb)
        nc.vector.reduce_max(nmaxm, scm, axis=AX.X, negate=True)
        exm = sbt("mex", (P, NMEM), f32, pool=sb)
        ssumm = sbt("mss", (P, 1), f32, pool=sb)
        nc.scalar.activation(exm, scm, AF.Exp, bias=nmaxm, accum_out=ssumm)
        recm = sbt("mrc", (P, 1), f32, pool=sb)
        nc.vector.reciprocal(recm, ssumm)
        Pm_s = sbt("Pm_s", (P, NMEM), bf16, pool=sb)
        nc.vector.tensor_scalar(Pm_s, exm, recm, dpow[:, 0:1], op0=OP.mult,
                                op1=OP.mult)

        pmt_ps = ps.tile([NMEM, P], bf16, tag="ps")
        nc.tensor.transpose(pmt_ps, Pm_s, ident)
        PmT = sbt("PmT", (NMEM, P), bf16, pool=sb)
        nc.vector.tensor_copy(PmT, pmt_ps)

        wtm_ps = pg.tile([P, P], f32, tag="pg")
        mm(wtm_ps, A_mT, PmT, start=True, stop=True)
        WTmM = sbt("WTmM", (P, P), bf16, pool=sb)
        nc.vector.tensor_mul(WTmM, wtm_ps, mdec2)

        MT = rd.tile([P, ND, P], mybir.dt.float8e4, tag="MT")
        for q in range(ND // 4):
            m_ps = pg.tile([P, 4, P], f32, tag="pg", name="mps%d" % q)
            for i in range(4):
                dc = 4 * q + i
                mm(m_ps[:, i, :], mv_c[:, dc * P:(dc + 1) * P], WTmM,
                   start=True, stop=False)
                mm(m_ps[:, i, :], Cm_bf[0:NMEM, dc * P:(dc + 1) * P], PmT,
                   start=False, stop=True)
            nc.vector.tensor_scalar_mul(MT[:, 4 * q:4 * q + 4, :], m_ps,
                                        1.0 / 16.0)

        # ---------------- state update (for next chunk)
        if c < NCH - 1:
            for jc in range(2):
                jsl = slice(jc * 512, (jc + 1) * 512)
                d_ps = po.tile([NREG, 512], f32, tag="po")
                mm(d_ps, A_r, rv_c[:, jsl], start=True, stop=True)
                nc.vector.tensor_add(Cr[:, jsl], Cr[:, jsl], d_ps)
                dm_ps = po.tile([NMEM, 512], f32, tag="po")
                mm(dm_ps, A_md, mv_c[:, jsl], start=True, stop=True)
                nc.vector.scalar_tensor_tensor(Cm[:, jsl], Cm[:, jsl], D128,
                                               dm_ps, op0=OP.mult, op1=OP.add)
            dt_ps = ps.tile([P, ND, NREG], f32, tag="ps")
            for dc in range(ND):
                mm(dt_ps[:, dc, :], rv_c[:, dc * P:(dc + 1) * P], A_r,
                   start=True, stop=True)
            nc.vector.tensor_add(CrT, CrT, dt_ps)
            dtm_ps = ps.tile([P, ND, NMEM], f32, tag="ps")
            for dc in range(ND):
                mm(dtm_ps[:, dc, :], mv_c[:, dc * P:(dc + 1) * P], A_md,
                   start=True, stop=True)
            nc.vector.scalar_tensor_tensor(CmT, CmT, D128, dtm_ps,
                                           op0=OP.mult, op1=OP.add)
            nc.vector.tensor_copy(Cr_bf, Cr)
            nc.vector.tensor_copy(CrT_bf, CrT)
            nc.vector.tensor_copy(Cm_bf, Cm)
            nc.vector.tensor_copy(CmT_bf, CmT)

        # ---------------- combine + layernorm
        xc = sbt("xc", (P, D), f32, pool=sb)
        ssums = sbt("ssums", (P, 2), f32, pool=sb)
        jsl0, jsl1 = slice(0, 512), slice(512, 1024)
        op0_ = po.tile([P, 512], f32, tag="po", name="op0")
        op1_ = po.tile([P, 512], f32, tag="po", name="op1")
        for dc in range(ND):
            mm(op0_, u_c[:, dc, :], whI[:, dc, jsl0], start=(dc == 0),
               stop=False)
            mm(op1_, u_c[:, dc, :], whI[:, dc, jsl1], start=(dc == 0),
               stop=False)
        if not zcombb:
            mm(op0_, ones_r[0:1, 0:P], combb[0:1, jsl0], start=False,
               stop=False)
            mm(op1_, ones_r[0:1, 0:P], combb[0:1, jsl1], start=False,
               stop=False)
        DR = mybir.MatmulPerfMode.DoubleRow
        for k in range(ND // 2):
            mm(op0_, RT[:, 2 * k:2 * k + 2, :], wr_[:, 2 * k:2 * k + 2, jsl0],
               start=False, stop=False, pm=DR)
            mm(op1_, RT[:, 2 * k:2 * k + 2, :], wr_[:, 2 * k:2 * k + 2, jsl1],
               start=False, stop=False, pm=DR)
        for k in range(ND // 2):
            mm(op0_, MT[:, 2 * k:2 * k + 2, :], wm_[:, 2 * k:2 * k + 2, jsl0],
               start=False, stop=(k == ND // 2 - 1), pm=DR)
            mm(op1_, MT[:, 2 * k:2 * k + 2, :], wm_[:, 2 * k:2 * k + 2, jsl1],
               start=False, stop=(k == ND // 2 - 1), pm=DR)
        for jc, o_ps in ((0, op0_), (1, op1_)):
            jsl = slice(jc * 512, (jc + 1) * 512)
            nc.vector.tensor_scalar(xc[:, jsl], o_ps, 1.0, None, op0=OP.mult,
                                    op1=OP.add,
                                    accum_out=ssums[:, jc:jc + 1])
        negmean = sbt("negmean", (P, 1), f32, pool=sb)
        nc.vector.tensor_scalar(negmean, ssums[:, 0:1], ssums[:, 1:2],
                                -1.0 / D, op0=OP.add, op1=OP.mult)
        nc.vector.tensor_scalar_add(xc, xc, negmean)
        vss = sbt("vss", (P, 2), f32, pool=sb)
        for jc in range(2):
            jsl = slice(jc * 512, (jc + 1) * 512)
            sqscr = sbt("sqscr", (P, 512), bf16, pool=sb)
            nc.scalar.activation(sqscr, xc[:, jsl], AF.Square,
                                 accum_out=vss[:, jc:jc + 1])
        var1 = sbt("var1", (P, 1), f32, pool=sb)
        nc.vector.tensor_scalar(var1, vss[:, 0:1], vss[:, 1:2], None,
                                op0=OP.add)
        nc.vector.tensor_scalar(var1, var1, 1.0 / D, 1e-5, op0=OP.mult,
                                op1=OP.add)
        sd = sbt("sd", (P, 1), f32, pool=sb)
        nc.scalar.activation(sd, var1, AF.Sqrt)
        rstd = sbt("rstd", (P, 1), f32, pool=sb)
        nc.vector.reciprocal(rstd, sd)
        if zln:
            nc.vector.tensor_scalar_mul(xc, xc, rstd)
        else:
            nc.vector.scalar_tensor_tensor(xc, xc, rstd, lng_rep,
                                           op0=OP.mult, op1=OP.mult)
            nc.vector.tensor_add(xc, xc, lnb_rep)
        nc.sync.dma_start(out_r3[:, c, :], xc)


# ---------------------------------------------------------------- host side
def _host_consts(is2: float):
    tau = np.arange(P, dtype=np.float64)
    maskUT = (tau[:, None] <= tau[None, :]).astype(np.float64)
    mdec = maskUT * DECAY ** (tau[None, :] - tau[:, None])
    mdec2 = maskUT * DECAY ** (-tau[:, None] - 1.0)
    dpowv = DECAY ** (tau[:, None] + 1.0)
    decvec = DECAY ** (P - 1.0 - tau[:, None])
    wdecprev = np.zeros((P, NCH))
    for c in range(NCH):
        wdecprev[:, c] = is2 * DECAY ** (T - 1.0 - (c * P + tau))
    return {
        "maskUT": maskUT.astype(F32), "mdec": mdec.astype(F32),
        "mdec2": mdec2.astype(F32), "dpow": dpowv.astype(F32),
        "decvec": decvec.astype(F32), "wdecprev": wdecprev.astype(F32),
        "prevmask": np.full((P, 1), is2, F32),
    }


def _host_weights(inputs):
    g = lambda k: np.asarray(inputs[k], np.float64)
    wcat = np.concatenate([g("reg_gate_w"), g("reg_addr_w"),
                           g("mem_gate_w"), g("mem_addr_w")], 0)  # (26, D)
    bcat = np.concatenate([g("reg_gate_b"), g("reg_addr_b"),
                           g("mem_gate_b"), g("mem_addr_b")], 0)  # (26,)
    comb = g("comb_w")
    W_h, W_r, W_m = comb[:, :D], comb[:, D:2 * D], comb[:, 2 * D:]
    bvq = np.concatenate([g("reg_val_b"), g("reg_q_b") * SCALE,
                          g("mem_val_b"), g("mem_q_b") * SCALE])[None, :]
    return {
        "wrvT": g("reg_val_w").T.astype(BF),
        "wrqT": (g("reg_q_w").T * SCALE).astype(BF),
        "wmvT": g("mem_val_w").T.astype(BF),
        "wmqT": (g("mem_q_w").T * SCALE).astype(BF),
        "whIT": (W_h + np.eye(D)).T.astype(BF),
        "wr8": (W_r.T * 16.0).astype(E4),
        "wm8": (W_m.T * 16.0).astype(E4),
        "wcatT": np.ascontiguousarray(wcat.T).astype(BF),
        "bcat": bcat[None, :].astype(BF),
        "bvq": bvq.astype(BF),
        "combb": g("comb_b")[None, :].astype(BF),
        "lng": g("ln_g")[None, :].astype(BF),
        "lnb": g("ln_b")[None, :].astype(BF),
    }


def host_in_maps(inputs):
    u = np.asarray(inputs["u"], F32)
    wmap = _host_weights(inputs)
    consts = [_host_consts(0.0), _host_consts(1.0)]
    zeros_bf = np.zeros((T, D), BF)
    in_maps = []
    for i in range(8):
        b, hf = i // 2, i % 2
        u_own = u[b, hf * T:(hf + 1) * T]
        m = dict(wmap)
        m.update(consts[hf])
        m["uT"] = np.ascontiguousarray(u_own.T).astype(BF)
        if hf:
            u_prev = u[b, :T]
            m["uprevT"] = np.ascontiguousarray(u_prev.T).astype(BF)
            m["uprev"] = u_prev.astype(BF)
        else:
            m["uprevT"] = zeros_bf
            m["uprev"] = zeros_bf
        in_maps.append(m)
    return in_maps


_NC_CACHE = {}


def zero_flags(inputs):
    g = lambda k: np.asarray(inputs[k])
    zbias = not (np.any(g("reg_val_b")) or np.any(g("reg_q_b"))
                 or np.any(g("mem_val_b")) or np.any(g("mem_q_b")))
    zcombb = not np.any(g("comb_b"))
    zln = (not np.any(g("ln_b"))) and bool(np.all(g("ln_g") == 1.0))
    return (bool(zbias), zcombb, zln)


def build_nc(flags=(False, False, False)):
    if flags in _NC_CACHE:
        return _NC_CACHE[flags]
    nc = bacc.Bacc("TRN2", target_bir_lowering=False, debug=False,
                   num_devices=8)
    ins = {name: nc.dram_tensor(name, list(shape), _dt(dt),
                                kind="ExternalInput").ap()
           for name, shape, dt in IN_SPECS}
    outs = {"out": nc.dram_tensor("out", [T, D], f32,
                                  kind="ExternalOutput").ap()}
    with tile.TileContext(nc) as tc:
        with ExitStack() as ctx:
            build_tile_kernel(ctx, tc, outs, ins, *flags)
    nc.compile()
    _NC_CACHE[flags] = nc
    return nc


def kernel(**inputs):
    from concourse import bass_utils
    nc = build_nc(zero_flags(inputs))
    in_maps = host_in_maps(inputs)
    res = bass_utils.run_bass_kernel_spmd(nc, in_maps, core_ids=list(range(8)))
    out = np.empty((B, L, D), F32)
    for i in range(8):
        b, hf = i // 2, i % 2
        out[b, hf * T:(hf + 1) * T] = np.asarray(res.results[i]["out"], F32)
    return out
